# revision 14
# baseline (speedup 1.0000x reference)
"""Trainium2 Bass kernel for nn_CrossAttentionPositionBridge.

Contract: kernel(**inputs) takes FULL unsharded inputs (as produced by
setup_inputs) and returns the FULL (4, 4096, 1024) float32 output.

Strategy (v2):
  - Each of the 4 rows is split at the first patch boundary >= 2048 into two
    chunks -> 8 chunks, one per NeuronCore.  Splitting at a patch boundary
    makes every patch fully contained in one chunk.  Chunks are zero-padded
    to P=2176 positions; local patch ids are padded with NP-1=383 (a dummy
    patch that only padded positions reference).
  - Ragged segment ops are matmuls against 0/1 selection matrices generated
    on-device with is_equal.  Patch ids are MONOTONE in position, so each
    128-position block only touches 1-2 of the three 128-patch blocks and
    each patch block only receives from a contiguous range of position
    blocks.  The host computes these windows (unioned over the 8 shards) and
    the kernel only emits the non-trivially-zero sel matmuls -- cutting the
    selection-matmul work to ~43% of the dense version.
  - All matmul operands are float16 (1 cycle/row on the PE at any free size;
    ints <= 2048 exact so the 0/1 sel masks and pid comparisons are exact).
    PSUM accumulation stays fp32; softmax statistics stay fp32 on the DVE.
    exp(score) values live in [e^-3, e^3] for this data regime, and
    1/denom is scaled by 256 with eps=0.01 so fp16 never over/underflows.
  - decode stage folded on host: out = gather(patch_heads @ (Wo2@Wv2@Wo).T).
  - x and the per-position k are never revisited: v and all gathered
    operands are kept SBUF-resident in fp16.
"""

import numpy as np

import concourse.bass as bass
import concourse.mybir as mybir
import concourse.tile as tile
from concourse import bacc, bass_utils
from concourse.bass import ts

B, S, D, H = 4, 4096, 1024, 16
HD = D // H
P = 2176           # padded chunk length
TB = P // 128      # 17 position blocks
NP = 384           # padded patch count
NB = NP // 128     # 3 patch blocks
DC = D // 128      # 8 feature chunks
N_CORES = 8

F32 = mybir.dt.float32
F16 = mybir.dt.float16

_PROG_CACHE = {}


def _build_body(nc, tc, aps, flags, meta, dbg=None, stages=frozenset(("head", "s4", "s67"))):
    """Emit the per-core kernel body into the TileContext."""
    from contextlib import ExitStack

    def dump(name, tile_ap):
        if dbg is not None and name in dbg:
            nc.sync.dma_start(dbg[name], tile_ap)

    winb = meta["winb"]          # tuple[tb] -> tuple of nb blocks touched
    span = meta["span"]          # tuple[nb] -> (tb_lo, tb_hi) inclusive

    x16, xT16, pid, iota_np, iota_col, invcnt = (
        aps["x16"], aps["xT16"], aps["pid"], aps["iota_np"], aps["iota_col"],
        aps["invcnt"])
    wqT, wkT, wvT, wfullT = aps["wqT"], aps["wkT"], aps["wvT"], aps["wfullT"]
    out = aps["out"]
    with_bq, with_bk, with_bv, with_bfull = (
        flags["bq"], flags["bk"], flags["bv"], flags["bfull"])

    x_r = x16.rearrange("(tb p) d -> p tb d", p=128)
    xT_r = xT16.rearrange("(dc p) t -> p dc t", p=128)
    pid_nat_r = pid.rearrange("(tb p) -> p tb", p=128)
    out_r = out.rearrange("(tb p) d -> p tb d", p=128)
    wq_r = wqT.rearrange("(dc p) d -> p dc d", p=128)
    wk_r = wkT.rearrange("(dc p) d -> p dc d", p=128)
    wv_r = wvT.rearrange("(dc p) d -> p dc d", p=128)
    wfull_r = wfullT.rearrange("(dc p) d -> p dc d", p=128)

    EQ = mybir.AluOpType.is_equal
    ADD = mybir.AluOpType.add
    MUL = mybir.AluOpType.mult

    with ExitStack() as ctx:
        # ---- long-lived tiles -------------------------------------------
        perm = ctx.enter_context(tc.tile_pool(name="perm", bufs=1))
        x_sb = perm.tile([128, TB, D], F16)          # resident x
        v_sb = perm.tile([128, TB, D], F16)          # resident v
        st_sb = perm.tile([128, TB, NP], F16)        # sel^T (t-part)
        sel_sb = perm.tile([128, NB, TB, 128], F16)  # sel (np-part)
        q_sb = perm.tile([128, NB, D], F16)
        qmT_sb = perm.tile([128, DC, NP], F16)
        upw_sb = perm.tile([128, DC, NP], F16)
        o2_sb = perm.tile([128, NB, D], F16)
        p16_sb = perm.tile([128, TB, H], F16)
        score_sb = perm.tile([128, TB, H], F32)
        invd16_sb = perm.tile([128, NB * H], F16)
        invcnt_repl = perm.tile([128, NP], F32)
        iota_np_repl = perm.tile([128, NP], F32)
        iota_col_sb = perm.tile([128, NB], F32)
        pid_nat = perm.tile([128, TB], F32)

        if dbg is not None:
            # debug dumps read whole tiles; zero the sparsely-written ones
            nc.gpsimd.memset(sel_sb[:], 0.0)
        nc.sync.dma_start(invcnt_repl[:], invcnt.partition_broadcast(128))
        nc.sync.dma_start(iota_np_repl[:], iota_np.partition_broadcast(128))
        nc.sync.dma_start(iota_col_sb[:], iota_col[:])
        nc.sync.dma_start(pid_nat[:], pid_nat_r[:])

        # x streamed per tb on the SP ring (17 slices into the perm tile)
        for tb in range(TB):
            nc.sync.dma_start(x_sb[:, tb, :], x_r[:, tb, :])

        # bias broadcasts (rarely used; zero biases skip these)
        bq_repl = bk_repl = bv_repl = bfull_repl = None
        if with_bq:
            bq_repl = perm.tile([128, D], F32)
            nc.sync.dma_start(bq_repl[:], aps["bq"].partition_broadcast(128))
        if with_bk:
            bk_repl = perm.tile([128, D], F32)
            nc.sync.dma_start(bk_repl[:], aps["bk"].partition_broadcast(128))
        if with_bv:
            bv_repl = perm.tile([128, D], F32)
            nc.sync.dma_start(bv_repl[:], aps["bv"].partition_broadcast(128))
        if with_bfull:
            bfull_repl = perm.tile([128, D], F32)
            nc.sync.dma_start(bfull_repl[:],
                              aps["bfull"].partition_broadcast(128))

        # ---- sel generation (DVE, fp16 out) -----------------------------
        with tc.tile_pool(name="pidr", bufs=1) as pidr:
            pid_repl = pidr.tile([128, P], F32)
            nc.sync.dma_start(pid_repl[:], pid.partition_broadcast(128))
            for tb in range(TB):
                nc.vector.tensor_tensor(
                    st_sb[:, tb, :],
                    pid_nat[:, tb:tb + 1].to_broadcast([128, NP]),
                    iota_np_repl[:], EQ)
            for tb in range(TB):
                for nb in winb[tb]:
                    nc.vector.tensor_tensor(
                        sel_sb[:, nb, tb, :],
                        iota_col_sb[:, nb:nb + 1].to_broadcast([128, 128]),
                        pid_repl[:, ts(tb, 128)], EQ)

            # ---- head: qm (windowed groups) + q-projection --------------
            with tc.tile_pool(name="wqp", bufs=1) as wqp, \
                 tc.tile_pool(name="psh", bufs=2, space="PSUM") as psh, \
                 tc.tile_pool(name="psq", bufs=2, space="PSUM") as psq:
                wq_sb = wqp.tile([128, DC, D], F16)
                nc.scalar.dma_start(wq_sb[:], wq_r[:])
                for nb in range(NB):
                    lo, hi = span[nb]
                    g0 = psh.tile([128, 512], F32, tag="qmg0")
                    g1 = psh.tile([128, 512], F32, tag="qmg1")
                    gt = (g0, g1)
                    for tb in range(lo, hi + 1):
                        for db in range(DC):
                            # one accumulation group per PSUM bank: start only
                            # on the bank's first write (pending-zero makes
                            # the other quarters' first writes overwrite)
                            nc.tensor.matmul(
                                gt[db // 4][:, ts(db % 4, 128)],
                                x_sb[:, tb, ts(db, 128)],
                                st_sb[:, tb, ts(nb, 128)],
                                start=(tb == lo and db % 4 == 0),
                                stop=(tb == hi and db % 4 == 3))
                    for db in range(DC):
                        nc.vector.tensor_mul(
                            qmT_sb[:, db, ts(nb, 128)],
                            gt[db // 4][:, ts(db % 4, 128)],
                            invcnt_repl[:, ts(nb, 128)])
                for nb in range(NB):
                    for hf in range(2):
                        q_ps = psq.tile([128, 512], F32, tag="q")
                        for db in range(DC):
                            nc.tensor.matmul(
                                q_ps[:], qmT_sb[:, db, ts(nb, 128)],
                                wq_sb[:, db, ts(hf, 512)],
                                start=(db == 0), stop=(db == DC - 1))
                        dst = q_sb[:, nb, ts(hf, 512)]
                        if with_bq:
                            nc.vector.tensor_tensor(
                                dst, q_ps[:], bq_repl[:, ts(hf, 512)], ADD)
                        else:
                            nc.scalar.copy(dst, q_ps[:])

        dump("qmT", qmT_sb[:])
        dump("q", q_sb[:])

        # ---- S4': k, v, scores, p, dn, w', patch-head accumulation ------
        if "s4" not in stages:
            return
        invd_dram = aps["invd_rt"]
        with tc.tile_pool(name="wkv", bufs=1) as wkv, \
             tc.tile_pool(name="xts", bufs=3) as xts, \
             tc.tile_pool(name="zs", bufs=3) as zs, \
             tc.tile_pool(name="qps", bufs=2) as qps, \
             tc.tile_pool(name="ws", bufs=2) as ws, \
             tc.tile_pool(name="psmm", bufs=2, space="PSUM") as psmm, \
             tc.tile_pool(name="psup", bufs=2, space="PSUM") as psup, \
             tc.tile_pool(name="psdn", bufs=1, space="PSUM") as psdn:
            wk_sb = wkv.tile([128, DC, D], F16)
            wv_sb = wkv.tile([128, DC, D], F16)
            nc.scalar.dma_start(wk_sb[:], wk_r[:])
            nc.scalar.dma_start(wv_sb[:], wv_r[:])
            dn_ps = psdn.tile([128, 512], F32, name="dn_ps")
            dn_first = (0, winb[0][0])
            dn_last = (TB - 1, winb[TB - 1][-1])
            upgrp = {}

            def emit_qkv(hf, tb):
                """PE: qp gather, k, v; chain: z -> score -> exp(p16)."""
                wtb = winb[tb]
                qp_ps = psmm.tile([128, 512], F32, tag="mm",
                                  name=f"qp{hf}_{tb}")
                for j, nb in enumerate(wtb):
                    nc.tensor.matmul(
                        qp_ps[:], sel_sb[:, nb, tb, :],
                        q_sb[:, nb, ts(hf, 512)],
                        start=(j == 0), stop=(j == len(wtb) - 1))
                qp_t = qps.tile([128, 512], F32, tag="qp")
                nc.scalar.copy(qp_t[:], qp_ps[:])

                xt_t = xts.tile([128, DC, 128], F16, tag="xt")
                nc.scalar.dma_start(xt_t[:], xT_r[:, :, ts(tb, 128)])

                k_ps = psmm.tile([128, 512], F32, tag="mm",
                                 name=f"k{hf}_{tb}")
                for db in range(DC):
                    nc.tensor.matmul(
                        k_ps[:], xt_t[:, db, :], wk_sb[:, db, ts(hf, 512)],
                        start=(db == 0), stop=(db == DC - 1))
                z_t = zs.tile([128, 512], F32, tag="z")
                if with_bk:
                    nc.vector.tensor_tensor(
                        z_t[:], k_ps[:], bk_repl[:, ts(hf, 512)], ADD)
                    nc.vector.tensor_mul(z_t[:], z_t[:], qp_t[:])
                else:
                    nc.vector.tensor_mul(z_t[:], k_ps[:], qp_t[:])
                nc.vector.tensor_reduce(
                    score_sb[:, tb, ts(hf, 8)],
                    z_t[:].rearrange("p (h e) -> p h e", e=HD),
                    mybir.AxisListType.X, ADD)
                nc.scalar.activation(
                    p16_sb[:, tb, ts(hf, 8)], score_sb[:, tb, ts(hf, 8)],
                    mybir.ActivationFunctionType.Exp,
                    scale=1.0 / float(HD) ** 0.5)

                v_ps = psmm.tile([128, 512], F32, tag="mm",
                                 name=f"v{hf}_{tb}")
                for db in range(DC):
                    nc.tensor.matmul(
                        v_ps[:], xt_t[:, db, :], wv_sb[:, db, ts(hf, 512)],
                        start=(db == 0), stop=(db == DC - 1))
                if with_bv:
                    nc.vector.tensor_tensor(
                        v_sb[:, tb, ts(hf, 512)], v_ps[:],
                        bv_repl[:, ts(hf, 512)], ADD)
                else:
                    nc.scalar.copy(v_sb[:, tb, ts(hf, 512)], v_ps[:])

            def emit_wup(hf, tb):
                """Lagged: w' = p16*v (unnormalized attn), patch-head accum;
                dn in the hf1 pass."""
                wtb = winb[tb]
                w_t = ws.tile([128, 8, HD], F16, tag="w")
                nc.vector.tensor_tensor(
                    w_t[:],
                    v_sb[:, tb, ts(hf, 512)].rearrange(
                        "p (h e) -> p h e", e=HD),
                    p16_sb[:, tb, ts(hf, 8), None].to_broadcast([128, 8, HD]),
                    MUL)
                w_f = w_t[:].rearrange("p h e -> p (h e)")
                for nb in wtb:
                    lo, hi = span[nb]
                    if tb == lo:
                        upgrp[(nb, hf)] = psup.tile(
                            [128, 512], F32, tag=f"up{hf}",
                            name=f"up{hf}_{nb}")
                    gt = upgrp[(nb, hf)]
                    for dq in range(4):
                        nc.tensor.matmul(
                            gt[:, ts(dq, 128)], w_f[:, ts(dq, 128)],
                            st_sb[:, tb, ts(nb, 128)],
                            start=(tb == lo and dq == 0),
                            stop=(tb == hi and dq == 3))
                    if tb == hi:
                        # raw drain (normalization applied later via M)
                        for dq in range(4):
                            nc.vector.tensor_copy(
                                upw_sb[:, hf * 4 + dq, ts(nb, 128)],
                                gt[:, ts(dq, 128)])
                if hf == 1:
                    for nb in wtb:
                        nc.tensor.matmul(
                            dn_ps[:, ts(nb, H)], st_sb[:, tb, ts(nb, 128)],
                            p16_sb[:, tb, :],
                            start=((tb, nb) == dn_first),
                            stop=((tb, nb) == dn_last))

            LAG = 2
            for hf in range(2):
                for tb in range(TB):
                    emit_qkv(hf, tb)
                    if tb >= LAG:
                        emit_wup(hf, tb - LAG)
                for tb in range(TB - LAG, TB):
                    emit_wup(hf, tb)

            # invd = clamp(1/dn) -> DRAM roundtrip for head-major reload
            dn_t = zs.tile([128, NB * H], F32, tag="dn")
            nc.vector.tensor_scalar_add(dn_t[:], dn_ps[:, :NB * H], 1e-30)
            inv_t = zs.tile([128, NB * H], F32, tag="inv")
            nc.vector.reciprocal(inv_t[:], dn_t[:])
            # clamp so empty-patch 1/eps stays fp16-finite
            nc.vector.tensor_scalar_min(invd16_sb[:], inv_t[:], 60000.0)
            nc.sync.dma_start(invd_dram, invd16_sb[:])

        dump("score", score_sb[:])
        dump("p16", p16_sb[:])
        dump("invd16", invd16_sb[:])
        dump("v", v_sb[:])
        dump("st", st_sb[:])
        dump("sel", sel_sb[:])

        # ---- tail: normalize patch heads, o2, per-position gather -------
        if "s67" not in stages:
            return
        invd_cols = invd_dram.rearrange("c h -> h c")
        with tc.tile_pool(name="wfp", bufs=1) as wfp, \
             tc.tile_pool(name="pso", bufs=2, space="PSUM") as pso, \
             tc.tile_pool(name="oc", bufs=4) as oc:
            wfull_sb = wfp.tile([128, DC, D], F16)
            nc.scalar.dma_start(wfull_sb[:], wfull_r[:])
            m_sb = wfp.tile([128, DC, NB, 128], F16)
            for db in range(DC):
                for u in range(2):
                    for nb in range(NB):
                        col = nb * H + 2 * db + u
                        nc.sync.dma_start(
                            m_sb[:][ts(u, 64), db, nb, :],
                            invd_cols[col].partition_broadcast(64))
            m_f = m_sb[:].rearrange("p dc nb c -> p dc (nb c)")
            for db in range(DC):
                nc.vector.tensor_mul(upw_sb[:, db, :], upw_sb[:, db, :],
                                     m_f[:, db, :])

            dump("upw", upw_sb[:])

            done_o2 = -1
            for nb in range(NB):
                for hf in range(2):
                    o2_ps = pso.tile([128, 512], F32, tag="o2")
                    for db in range(DC):
                        nc.tensor.matmul(
                            o2_ps[:], upw_sb[:, db, ts(nb, 128)],
                            wfull_sb[:, db, ts(hf, 512)],
                            start=(db == 0), stop=(db == DC - 1))
                    dst = o2_sb[:, nb, ts(hf, 512)]
                    if with_bfull:
                        nc.vector.tensor_tensor(
                            dst, o2_ps[:], bfull_repl[:, ts(hf, 512)], ADD)
                    else:
                        nc.scalar.copy(dst, o2_ps[:])
                # emit gathers for all position blocks whose windows are ready
                for tb in range(TB):
                    if winb[tb][-1] != nb:
                        continue
                    wtb = winb[tb]
                    for hf in range(2):
                        o_ps = pso.tile([128, 512], F32, tag="o")
                        for j, nbb in enumerate(wtb):
                            nc.tensor.matmul(
                                o_ps[:], sel_sb[:, nbb, tb, :],
                                o2_sb[:, nbb, ts(hf, 512)],
                                start=(j == 0), stop=(j == len(wtb) - 1))
                        oc_t = oc.tile([128, 512], F16, tag="oc")
                        if hf == 0:
                            nc.vector.tensor_copy(oc_t[:], o_ps[:])
                            nc.sync.dma_start(out_r[:, tb, ts(hf, 512)],
                                              oc_t[:])
                        else:
                            nc.scalar.copy(oc_t[:], o_ps[:])
                            nc.gpsimd.dma_start(out_r[:, tb, ts(hf, 512)],
                                                oc_t[:])

        dump("o2", o2_sb[:])


def _build_program(flags, meta, loop_reps=None,
                   stages=frozenset(("head", "s4", "s67"))):
    nc = bacc.Bacc("TRN2", target_bir_lowering=False, debug=False)
    aps = {}
    aps["x16"] = nc.dram_tensor("x16", [P, D], F16, kind="ExternalInput").ap()
    aps["xT16"] = nc.dram_tensor("xT16", [D, P], F16,
                                 kind="ExternalInput").ap()
    aps["pid"] = nc.dram_tensor("pid", [P], F32, kind="ExternalInput").ap()
    aps["iota_np"] = nc.dram_tensor("iota_np", [NP], F32,
                                    kind="ExternalInput").ap()
    aps["iota_col"] = nc.dram_tensor("iota_col", [128, NB], F32,
                                     kind="ExternalInput").ap()
    aps["invcnt"] = nc.dram_tensor("invcnt", [NP], F32,
                                   kind="ExternalInput").ap()
    for w in ("wqT", "wkT", "wvT", "wfullT"):
        aps[w] = nc.dram_tensor(w, [D, D], F16, kind="ExternalInput").ap()
    aps["invd_rt"] = nc.dram_tensor("invd_rt", [128, NB * H], F16).ap()
    for b in ("bq", "bk", "bv", "bfull"):
        if flags[b]:
            aps[b] = nc.dram_tensor(b, [D], F32, kind="ExternalInput").ap()
    if loop_reps is not None:
        # Timing build: the big output stays in internal DRAM so the host
        # only ships a tiny donated zero buffer per timed call.
        aps["out"] = nc.dram_tensor("out_scratch", [P, D], F16).ap()
        dummy = nc.dram_tensor("out", [1, 1], F32, kind="ExternalOutput").ap()
    else:
        aps["out"] = nc.dram_tensor("out", [P, D], F16,
                                    kind="ExternalOutput").ap()

    with tile.TileContext(nc) as tc:
        if loop_reps is not None:
            with tc.For_i(0, loop_reps, 1):
                _build_body(nc, tc, aps, flags, meta, stages=stages)
            with tc.tile_pool(name="dum", bufs=1) as dum:
                d_t = dum.tile([1, 1], F32)
                nc.vector.memset(d_t[:], 0.0)
                nc.sync.dma_start(dummy[:], d_t[:])
        else:
            _build_body(nc, tc, aps, flags, meta, stages=stages)
    nc.compile()
    return nc


def get_program(flags=None, meta=None, loop_reps=None,
                stages=frozenset(("head", "s4", "s67"))):
    if flags is None:
        flags = {"bq": False, "bk": False, "bv": False, "bfull": False}
    key = (tuple(sorted(flags.items())), meta["winb"], meta["span"], loop_reps,
           stages)
    if key not in _PROG_CACHE:
        _PROG_CACHE[key] = _build_program(flags, meta, loop_reps, stages)
    return _PROG_CACHE[key]


def _make_shards(patch_boundaries):
    pb = np.asarray(patch_boundaries)
    shards = []
    for b in range(pb.shape[0]):
        bnd = (pb[b] != 0).astype(np.int64)
        pid = np.cumsum(bnd) - bnd[0]
        bpos = np.nonzero(bnd)[0]
        cand = bpos[bpos >= S // 2]
        split = int(cand[0]) if len(cand) else S
        for (t0, t1) in ((0, split), (split, S)):
            L = t1 - t0
            assert L <= P, f"chunk length {L} exceeds padded size {P}"
            pad_pid = np.full(P, NP - 1, np.int64)
            if L:
                lpid = pid[t0:t1] - pid[t0]
                assert lpid[-1] + 1 <= NP - 1, "too many patches in chunk"
                pad_pid[:L] = lpid
            cnt = np.bincount(pad_pid[:L], minlength=NP).astype(np.float32)
            invcnt = np.zeros(NP, np.float32)
            nz = cnt > 0
            invcnt[nz] = 1.0 / cnt[nz]
            invcnt[NP - 1] = 0.0
            shards.append(dict(row=b, t0=t0, L=L, pid=pad_pid, invcnt=invcnt))
    return shards


def _make_meta(shards):
    """Union (over shards) of position-block <-> patch-block adjacency."""
    winb = [set() for _ in range(TB)]
    span = [set() for _ in range(NB)]
    for sh in shards:
        pp = sh["pid"]
        for tb in range(TB):
            blk = pp[tb * 128:(tb + 1) * 128]
            for nb in range(int(blk.min()) // 128, int(blk.max()) // 128 + 1):
                winb[tb].add(nb)
                span[nb].add(tb)
    for nb in range(NB):
        s = span[nb]
        assert s and s == set(range(min(s), max(s) + 1)), \
            f"patch block {nb} has non-contiguous tb span {sorted(s)}"
    return {
        "winb": tuple(tuple(sorted(w)) for w in winb),
        "span": tuple((min(s), max(s)) for s in span),
    }


def prepare_in_maps(byte_repr, Wq, bq, Wk, bk, Wv, bv, Wo, bo, Wv2, bv2,
                    Wo2, bo2, patch_boundaries):
    """Host-side sharding/marshalling: (shards, in_maps, flags, meta)."""
    byte_repr = np.asarray(byte_repr, np.float32)
    shards = _make_shards(patch_boundaries)
    meta = _make_meta(shards)
    Wo = np.asarray(Wo, np.float64)
    Wv2 = np.asarray(Wv2, np.float64)
    Wo2 = np.asarray(Wo2, np.float64)
    wfull = Wo2 @ (Wv2 @ Wo)
    bfull = (Wo2 @ (Wv2 @ np.asarray(bo, np.float64)
                    + np.asarray(bv2, np.float64))
             + np.asarray(bo2, np.float64))
    flags = {
        "bq": bool(np.any(np.asarray(bq))),
        "bk": bool(np.any(np.asarray(bk))),
        "bv": bool(np.any(np.asarray(bv))),
        "bfull": bool(np.any(bfull)),
    }
    wqT = np.ascontiguousarray(np.asarray(Wq, np.float32).T.astype(np.float16))
    wkT = np.ascontiguousarray(np.asarray(Wk, np.float32).T.astype(np.float16))
    wvT = np.ascontiguousarray(np.asarray(Wv, np.float32).T.astype(np.float16))
    wfullT = np.ascontiguousarray(wfull.T.astype(np.float16))
    iota_np = np.arange(NP, dtype=np.float32)
    iota_col = (np.arange(128, dtype=np.float32)[:, None]
                + 128.0 * np.arange(NB, dtype=np.float32)[None, :])
    iota_col = np.ascontiguousarray(iota_col)

    in_maps = []
    for sh in shards:
        xc = np.zeros((P, D), np.float16)
        if sh["L"]:
            xc[:sh["L"]] = byte_repr[sh["row"],
                                     sh["t0"]:sh["t0"] + sh["L"]].astype(
                                         np.float16)
        m = {
            "x16": xc,
            "xT16": np.ascontiguousarray(xc.T),
            "pid": sh["pid"].astype(np.float32),
            "iota_np": iota_np,
            "iota_col": iota_col,
            "invcnt": sh["invcnt"],
            "wqT": wqT, "wkT": wkT, "wvT": wvT, "wfullT": wfullT,
        }
        if flags["bq"]:
            m["bq"] = np.asarray(bq, np.float32)
        if flags["bk"]:
            m["bk"] = np.asarray(bk, np.float32)
        if flags["bv"]:
            m["bv"] = np.asarray(bv, np.float32)
        if flags["bfull"]:
            m["bfull"] = bfull.astype(np.float32)
        in_maps.append(m)
    return shards, in_maps, flags, meta


def kernel(byte_repr, Wq, bq, Wk, bk, Wv, bv, Wo, bo, Wv2, bv2, Wo2, bo2,
           patch_boundaries):
    shards, in_maps, flags, meta = prepare_in_maps(
        byte_repr, Wq, bq, Wk, bk, Wv, bv, Wo, bo, Wv2, bv2, Wo2, bo2,
        patch_boundaries)
    nc = get_program(flags, meta)
    res = bass_utils.run_bass_kernel_spmd(nc, in_maps, list(range(N_CORES)))
    out = np.zeros((B, S, D), np.float32)
    for sh, r in zip(shards, res.results):
        if sh["L"]:
            out[sh["row"], sh["t0"]:sh["t0"] + sh["L"]] = (
                r["out"][:sh["L"]].astype(np.float32))
    return out


# revision 15
# speedup vs baseline: 2.2881x; 2.2881x over previous
"""Trainium2 Bass kernel for nn_CrossAttentionPositionBridge.

Contract: kernel(**inputs) takes FULL unsharded inputs (as produced by
setup_inputs) and returns the FULL (4, 4096, 1024) float32 output.

Strategy (v2):
  - Each of the 4 rows is split at the first patch boundary >= 2048 into two
    chunks -> 8 chunks, one per NeuronCore.  Splitting at a patch boundary
    makes every patch fully contained in one chunk.  Chunks are zero-padded
    to P=2176 positions; local patch ids are padded with NP-1=383 (a dummy
    patch that only padded positions reference).
  - Ragged segment ops are matmuls against 0/1 selection matrices generated
    on-device with is_equal.  Patch ids are MONOTONE in position, so each
    128-position block only touches 1-2 of the three 128-patch blocks and
    each patch block only receives from a contiguous range of position
    blocks.  The host computes these windows (unioned over the 8 shards) and
    the kernel only emits the non-trivially-zero sel matmuls -- cutting the
    selection-matmul work to ~43% of the dense version.
  - All matmul operands are float16 (1 cycle/row on the PE at any free size;
    ints <= 2048 exact so the 0/1 sel masks and pid comparisons are exact).
    PSUM accumulation stays fp32; softmax statistics stay fp32 on the DVE.
    exp(score) values live in [e^-3, e^3] for this data regime, and
    1/denom is scaled by 256 with eps=0.01 so fp16 never over/underflows.
  - decode stage folded on host: out = gather(patch_heads @ (Wo2@Wv2@Wo).T).
  - x and the per-position k are never revisited: v and all gathered
    operands are kept SBUF-resident in fp16.
"""

import numpy as np

import concourse.bass as bass
import concourse.mybir as mybir
import concourse.tile as tile
from concourse import bacc, bass_utils
from concourse.bass import ts

B, S, D, H = 4, 4096, 1024, 16
HD = D // H
P = 2176           # padded chunk length
TB = P // 128      # 17 position blocks
NP = 384           # padded patch count
NB = NP // 128     # 3 patch blocks
DC = D // 128      # 8 feature chunks
N_CORES = 8

F32 = mybir.dt.float32
F16 = mybir.dt.float16

_PROG_CACHE = {}


def _build_body(nc, tc, aps, flags, meta, dbg=None, stages=frozenset(("head", "s4", "s67"))):
    """Emit the per-core kernel body into the TileContext."""
    from contextlib import ExitStack

    def dump(name, tile_ap):
        if dbg is not None and name in dbg:
            nc.sync.dma_start(dbg[name], tile_ap)

    winb = meta["winb"]          # tuple[tb] -> tuple of nb blocks touched
    span = meta["span"]          # tuple[nb] -> (tb_lo, tb_hi) inclusive

    x16, xT16, pid, iota_np, iota_col, invcnt = (
        aps["x16"], aps["xT16"], aps["pid"], aps["iota_np"], aps["iota_col"],
        aps["invcnt"])
    wqT, wkT, wvT, wfullT = aps["wqT"], aps["wkT"], aps["wvT"], aps["wfullT"]
    out = aps["out"]
    with_bq, with_bk, with_bv, with_bfull = (
        flags["bq"], flags["bk"], flags["bv"], flags["bfull"])

    x_r = x16.rearrange("(tb p) d -> p tb d", p=128)
    xT_r = xT16.rearrange("(dc p) t -> p dc t", p=128)
    pid_nat_r = pid.rearrange("(tb p) -> p tb", p=128)
    out_r = out.rearrange("(tb p) d -> p tb d", p=128)
    wq_r = wqT.rearrange("(dc p) d -> p dc d", p=128)
    wk_r = wkT.rearrange("(dc p) d -> p dc d", p=128)
    wv_r = wvT.rearrange("(dc p) d -> p dc d", p=128)
    wfull_r = wfullT.rearrange("(dc p) d -> p dc d", p=128)

    EQ = mybir.AluOpType.is_equal
    ADD = mybir.AluOpType.add
    MUL = mybir.AluOpType.mult

    with ExitStack() as ctx:
        # ---- long-lived tiles -------------------------------------------
        perm = ctx.enter_context(tc.tile_pool(name="perm", bufs=1))
        x_sb = perm.tile([128, TB, D], F16)          # resident x
        v_sb = perm.tile([128, TB, D], F16)          # resident v
        st_sb = perm.tile([128, TB, NP], F16)        # sel^T (t-part)
        sel_sb = perm.tile([128, NB, TB, 128], F16)  # sel (np-part)
        q_sb = perm.tile([128, NB, D], F16)
        qmT_sb = perm.tile([128, DC, NP], F16)
        upw_sb = perm.tile([128, DC, NP], F16)
        o2_sb = perm.tile([128, NB, D], F16)
        p16_sb = perm.tile([128, TB, H], F16)
        score_sb = perm.tile([128, TB, H], F32)
        invd16_sb = perm.tile([16, NP], F16)
        invcnt_repl = perm.tile([128, NP], F32)
        iota_np_repl = perm.tile([128, NP], F32)
        iota_col_sb = perm.tile([128, NB], F32)
        pid_nat = perm.tile([128, TB], F32)

        if dbg is not None:
            # debug dumps read whole tiles; zero the sparsely-written ones
            nc.gpsimd.memset(sel_sb[:], 0.0)
        nc.sync.dma_start(invcnt_repl[:], invcnt.partition_broadcast(128))
        nc.sync.dma_start(iota_np_repl[:], iota_np.partition_broadcast(128))
        nc.sync.dma_start(iota_col_sb[:], iota_col[:])
        nc.sync.dma_start(pid_nat[:], pid_nat_r[:])

        # x streamed per tb on the SP ring (17 slices into the perm tile)
        for tb in range(TB):
            nc.sync.dma_start(x_sb[:, tb, :], x_r[:, tb, :])

        # bias broadcasts (rarely used; zero biases skip these)
        bq_repl = bk_repl = bv_repl = bfull_repl = None
        if with_bq:
            bq_repl = perm.tile([128, D], F32)
            nc.sync.dma_start(bq_repl[:], aps["bq"].partition_broadcast(128))
        if with_bk:
            bk_repl = perm.tile([128, D], F32)
            nc.sync.dma_start(bk_repl[:], aps["bk"].partition_broadcast(128))
        if with_bv:
            bv_repl = perm.tile([128, D], F32)
            nc.sync.dma_start(bv_repl[:], aps["bv"].partition_broadcast(128))
        if with_bfull:
            bfull_repl = perm.tile([128, D], F32)
            nc.sync.dma_start(bfull_repl[:],
                              aps["bfull"].partition_broadcast(128))

        # ---- sel generation (DVE, fp16 out) -----------------------------
        with tc.tile_pool(name="pidr", bufs=1) as pidr:
            pid_repl = pidr.tile([128, P], F32)
            nc.sync.dma_start(pid_repl[:], pid.partition_broadcast(128))
            for tb in range(TB):
                nc.vector.tensor_tensor(
                    st_sb[:, tb, :],
                    pid_nat[:, tb:tb + 1].to_broadcast([128, NP]),
                    iota_np_repl[:], EQ)
            for tb in range(TB):
                for nb in winb[tb]:
                    nc.vector.tensor_tensor(
                        sel_sb[:, nb, tb, :],
                        iota_col_sb[:, nb:nb + 1].to_broadcast([128, 128]),
                        pid_repl[:, ts(tb, 128)], EQ)

            # ---- head: qm (windowed groups) + q-projection --------------
            with tc.tile_pool(name="wqp", bufs=1) as wqp, \
                 tc.tile_pool(name="psh", bufs=2, space="PSUM") as psh, \
                 tc.tile_pool(name="psq", bufs=2, space="PSUM") as psq:
                wq_sb = wqp.tile([128, DC, D], F16)
                nc.scalar.dma_start(wq_sb[:], wq_r[:])
                for nb in range(NB):
                    lo, hi = span[nb]
                    g0 = psh.tile([128, 512], F32, tag="qmg0")
                    g1 = psh.tile([128, 512], F32, tag="qmg1")
                    gt = (g0, g1)
                    for tb in range(lo, hi + 1):
                        for db in range(DC):
                            # one accumulation group per PSUM bank: start only
                            # on the bank's first write (pending-zero makes
                            # the other quarters' first writes overwrite)
                            nc.tensor.matmul(
                                gt[db // 4][:, ts(db % 4, 128)],
                                x_sb[:, tb, ts(db, 128)],
                                st_sb[:, tb, ts(nb, 128)],
                                start=(tb == lo and db % 4 == 0),
                                stop=(tb == hi and db % 4 == 3))
                    for db in range(DC):
                        nc.vector.tensor_mul(
                            qmT_sb[:, db, ts(nb, 128)],
                            gt[db // 4][:, ts(db % 4, 128)],
                            invcnt_repl[:, ts(nb, 128)])
                for nb in range(NB):
                    for hf in range(2):
                        q_ps = psq.tile([128, 512], F32, tag="q")
                        for db in range(DC):
                            nc.tensor.matmul(
                                q_ps[:], qmT_sb[:, db, ts(nb, 128)],
                                wq_sb[:, db, ts(hf, 512)],
                                start=(db == 0), stop=(db == DC - 1))
                        dst = q_sb[:, nb, ts(hf, 512)]
                        if with_bq:
                            nc.vector.tensor_tensor(
                                dst, q_ps[:], bq_repl[:, ts(hf, 512)], ADD)
                        else:
                            nc.scalar.copy(dst, q_ps[:])

        dump("qmT", qmT_sb[:])
        dump("q", q_sb[:])

        # ---- S4': k, v, scores, p, dn, w', patch-head accumulation ------
        if "s4" not in stages:
            return
        invd_dram = aps["invd_rt"]
        with tc.tile_pool(name="wkv", bufs=1) as wkv, \
             tc.tile_pool(name="xts", bufs=3) as xts, \
             tc.tile_pool(name="zs", bufs=3) as zs, \
             tc.tile_pool(name="qps", bufs=2) as qps, \
             tc.tile_pool(name="ws", bufs=2) as ws, \
             tc.tile_pool(name="psmm", bufs=2, space="PSUM") as psmm, \
             tc.tile_pool(name="psup", bufs=2, space="PSUM") as psup, \
             tc.tile_pool(name="psdn", bufs=1, space="PSUM") as psdn:
            wk_sb = wkv.tile([128, DC, D], F16)
            wv_sb = wkv.tile([128, DC, D], F16)
            nc.scalar.dma_start(wk_sb[:], wk_r[:])
            nc.scalar.dma_start(wv_sb[:], wv_r[:])
            dn_ps = psdn.tile([16, NP], F32, name="dn_ps")
            dn_first = (0, winb[0][0])
            dn_last = (TB - 1, winb[TB - 1][-1])
            upgrp = {}

            def emit_qkv(hf, tb):
                """PE: qp gather, k, v; chain: z -> score -> exp(p16)."""
                wtb = winb[tb]
                qp_ps = psmm.tile([128, 512], F32, tag="mm",
                                  name=f"qp{hf}_{tb}")
                for j, nb in enumerate(wtb):
                    nc.tensor.matmul(
                        qp_ps[:], sel_sb[:, nb, tb, :],
                        q_sb[:, nb, ts(hf, 512)],
                        start=(j == 0), stop=(j == len(wtb) - 1))
                qp_t = qps.tile([128, 512], F32, tag="qp")
                nc.scalar.copy(qp_t[:], qp_ps[:])

                xt_t = xts.tile([128, DC, 128], F16, tag="xt")
                nc.scalar.dma_start(xt_t[:], xT_r[:, :, ts(tb, 128)])

                k_ps = psmm.tile([128, 512], F32, tag="mm",
                                 name=f"k{hf}_{tb}")
                for db in range(DC):
                    nc.tensor.matmul(
                        k_ps[:], xt_t[:, db, :], wk_sb[:, db, ts(hf, 512)],
                        start=(db == 0), stop=(db == DC - 1))
                z_t = zs.tile([128, 512], F32, tag="z")
                if with_bk:
                    nc.vector.tensor_tensor(
                        z_t[:], k_ps[:], bk_repl[:, ts(hf, 512)], ADD)
                    nc.vector.tensor_mul(z_t[:], z_t[:], qp_t[:])
                else:
                    nc.vector.tensor_mul(z_t[:], k_ps[:], qp_t[:])
                nc.vector.tensor_reduce(
                    score_sb[:, tb, ts(hf, 8)],
                    z_t[:].rearrange("p (h e) -> p h e", e=HD),
                    mybir.AxisListType.X, ADD)
                nc.scalar.activation(
                    p16_sb[:, tb, ts(hf, 8)], score_sb[:, tb, ts(hf, 8)],
                    mybir.ActivationFunctionType.Exp,
                    scale=1.0 / float(HD) ** 0.5)

                v_ps = psmm.tile([128, 512], F32, tag="mm",
                                 name=f"v{hf}_{tb}")
                for db in range(DC):
                    nc.tensor.matmul(
                        v_ps[:], xt_t[:, db, :], wv_sb[:, db, ts(hf, 512)],
                        start=(db == 0), stop=(db == DC - 1))
                if with_bv:
                    nc.vector.tensor_tensor(
                        v_sb[:, tb, ts(hf, 512)], v_ps[:],
                        bv_repl[:, ts(hf, 512)], ADD)
                else:
                    nc.scalar.copy(v_sb[:, tb, ts(hf, 512)], v_ps[:])

            def emit_wup(hf, tb):
                """Lagged: w' = p16*v (unnormalized attn), patch-head accum;
                dn in the hf1 pass."""
                wtb = winb[tb]
                w_t = ws.tile([128, 8, HD], F16, tag="w")
                nc.vector.tensor_tensor(
                    w_t[:],
                    v_sb[:, tb, ts(hf, 512)].rearrange(
                        "p (h e) -> p h e", e=HD),
                    p16_sb[:, tb, ts(hf, 8), None].to_broadcast([128, 8, HD]),
                    MUL)
                w_f = w_t[:].rearrange("p h e -> p (h e)")
                for nb in wtb:
                    lo, hi = span[nb]
                    if tb == lo:
                        upgrp[(nb, hf)] = psup.tile(
                            [128, 512], F32, tag=f"up{hf}",
                            name=f"up{hf}_{nb}")
                    gt = upgrp[(nb, hf)]
                    for dq in range(4):
                        nc.tensor.matmul(
                            gt[:, ts(dq, 128)], w_f[:, ts(dq, 128)],
                            st_sb[:, tb, ts(nb, 128)],
                            start=(tb == lo and dq == 0),
                            stop=(tb == hi and dq == 3))
                    if tb == hi:
                        # raw drain (normalization applied later via M)
                        for dq in range(4):
                            nc.vector.tensor_copy(
                                upw_sb[:, hf * 4 + dq, ts(nb, 128)],
                                gt[:, ts(dq, 128)])
                if hf == 1:
                    for nb in wtb:
                        # transposed denominator: out [16 heads, 128 patches]
                        nc.tensor.matmul(
                            dn_ps[:, ts(nb, 128)], p16_sb[:, tb, :],
                            st_sb[:, tb, ts(nb, 128)],
                            start=((tb, nb) == dn_first),
                            stop=((tb, nb) == dn_last))

            LAG = 2
            for hf in range(2):
                for tb in range(TB):
                    emit_qkv(hf, tb)
                    if tb >= LAG:
                        emit_wup(hf, tb - LAG)
                for tb in range(TB - LAG, TB):
                    emit_wup(hf, tb)

            # invd = clamp(1/dn), head-major -> tiny DRAM bounce for the
            # per-head broadcast reload in the tail
            dn_t = zs.tile([16, NP], F32, tag="dn")
            nc.vector.tensor_scalar_add(dn_t[:], dn_ps[:], 1e-30)
            inv_t = zs.tile([16, NP], F32, tag="inv")
            nc.vector.reciprocal(inv_t[:], dn_t[:])
            # clamp so empty-patch 1/eps stays fp16-finite
            nc.vector.tensor_scalar_min(invd16_sb[:], inv_t[:], 60000.0)
            nc.sync.dma_start(invd_dram, invd16_sb[:])

        dump("score", score_sb[:])
        dump("p16", p16_sb[:])
        dump("invd16", invd16_sb[:])
        dump("v", v_sb[:])
        dump("st", st_sb[:])
        dump("sel", sel_sb[:])

        # ---- tail: normalize patch heads, o2, per-position gather -------
        if "s67" not in stages:
            return
        with tc.tile_pool(name="wfp", bufs=1) as wfp, \
             tc.tile_pool(name="pso", bufs=2, space="PSUM") as pso, \
             tc.tile_pool(name="oc", bufs=4) as oc:
            wfull_sb = wfp.tile([128, DC, D], F16)
            nc.scalar.dma_start(wfull_sb[:], wfull_r[:])
            m_sb = wfp.tile([128, DC, NP], F16)
            for db in range(DC):
                for u in range(2):
                    nc.sync.dma_start(
                        m_sb[:][ts(u, 64), db, :],
                        invd_dram[2 * db + u].partition_broadcast(64))
            for db in range(DC):
                nc.vector.tensor_mul(upw_sb[:, db, :], upw_sb[:, db, :],
                                     m_sb[:, db, :])

            dump("upw", upw_sb[:])

            done_o2 = -1
            for nb in range(NB):
                for hf in range(2):
                    o2_ps = pso.tile([128, 512], F32, tag="o2")
                    for db in range(DC):
                        nc.tensor.matmul(
                            o2_ps[:], upw_sb[:, db, ts(nb, 128)],
                            wfull_sb[:, db, ts(hf, 512)],
                            start=(db == 0), stop=(db == DC - 1))
                    dst = o2_sb[:, nb, ts(hf, 512)]
                    if with_bfull:
                        nc.vector.tensor_tensor(
                            dst, o2_ps[:], bfull_repl[:, ts(hf, 512)], ADD)
                    else:
                        nc.scalar.copy(dst, o2_ps[:])
                # emit gathers for all position blocks whose windows are ready
                for tb in range(TB):
                    if winb[tb][-1] != nb:
                        continue
                    wtb = winb[tb]
                    for hf in range(2):
                        o_ps = pso.tile([128, 512], F32, tag="o")
                        for j, nbb in enumerate(wtb):
                            nc.tensor.matmul(
                                o_ps[:], sel_sb[:, nbb, tb, :],
                                o2_sb[:, nbb, ts(hf, 512)],
                                start=(j == 0), stop=(j == len(wtb) - 1))
                        oc_t = oc.tile([128, 512], F16, tag="oc")
                        if hf == 0:
                            nc.vector.tensor_copy(oc_t[:], o_ps[:])
                            nc.sync.dma_start(out_r[:, tb, ts(hf, 512)],
                                              oc_t[:])
                        else:
                            nc.scalar.copy(oc_t[:], o_ps[:])
                            nc.scalar.dma_start(out_r[:, tb, ts(hf, 512)],
                                                oc_t[:])

        dump("o2", o2_sb[:])


def _build_program(flags, meta, loop_reps=None,
                   stages=frozenset(("head", "s4", "s67"))):
    nc = bacc.Bacc("TRN2", target_bir_lowering=False, debug=False)
    aps = {}
    aps["x16"] = nc.dram_tensor("x16", [P, D], F16, kind="ExternalInput").ap()
    aps["xT16"] = nc.dram_tensor("xT16", [D, P], F16,
                                 kind="ExternalInput").ap()
    aps["pid"] = nc.dram_tensor("pid", [P], F32, kind="ExternalInput").ap()
    aps["iota_np"] = nc.dram_tensor("iota_np", [NP], F32,
                                    kind="ExternalInput").ap()
    aps["iota_col"] = nc.dram_tensor("iota_col", [128, NB], F32,
                                     kind="ExternalInput").ap()
    aps["invcnt"] = nc.dram_tensor("invcnt", [NP], F32,
                                   kind="ExternalInput").ap()
    for w in ("wqT", "wkT", "wvT", "wfullT"):
        aps[w] = nc.dram_tensor(w, [D, D], F16, kind="ExternalInput").ap()
    aps["invd_rt"] = nc.dram_tensor("invd_rt", [16, NP], F16).ap()
    for b in ("bq", "bk", "bv", "bfull"):
        if flags[b]:
            aps[b] = nc.dram_tensor(b, [D], F32, kind="ExternalInput").ap()
    if loop_reps is not None:
        # Timing build: the big output stays in internal DRAM so the host
        # only ships a tiny donated zero buffer per timed call.
        aps["out"] = nc.dram_tensor("out_scratch", [P, D], F16).ap()
        dummy = nc.dram_tensor("out", [1, 1], F32, kind="ExternalOutput").ap()
    else:
        aps["out"] = nc.dram_tensor("out", [P, D], F16,
                                    kind="ExternalOutput").ap()

    with tile.TileContext(nc) as tc:
        if loop_reps is not None:
            with tc.For_i(0, loop_reps, 1):
                _build_body(nc, tc, aps, flags, meta, stages=stages)
            with tc.tile_pool(name="dum", bufs=1) as dum:
                d_t = dum.tile([1, 1], F32)
                nc.vector.memset(d_t[:], 0.0)
                nc.sync.dma_start(dummy[:], d_t[:])
        else:
            _build_body(nc, tc, aps, flags, meta, stages=stages)
    nc.compile()
    return nc


def get_program(flags=None, meta=None, loop_reps=None,
                stages=frozenset(("head", "s4", "s67"))):
    if flags is None:
        flags = {"bq": False, "bk": False, "bv": False, "bfull": False}
    key = (tuple(sorted(flags.items())), meta["winb"], meta["span"], loop_reps,
           stages)
    if key not in _PROG_CACHE:
        _PROG_CACHE[key] = _build_program(flags, meta, loop_reps, stages)
    return _PROG_CACHE[key]


def _make_shards(patch_boundaries):
    pb = np.asarray(patch_boundaries)
    shards = []
    for b in range(pb.shape[0]):
        bnd = (pb[b] != 0).astype(np.int64)
        pid = np.cumsum(bnd) - bnd[0]
        bpos = np.nonzero(bnd)[0]
        cand = bpos[bpos >= S // 2]
        split = int(cand[0]) if len(cand) else S
        for (t0, t1) in ((0, split), (split, S)):
            L = t1 - t0
            assert L <= P, f"chunk length {L} exceeds padded size {P}"
            pad_pid = np.full(P, NP - 1, np.int64)
            if L:
                lpid = pid[t0:t1] - pid[t0]
                assert lpid[-1] + 1 <= NP - 1, "too many patches in chunk"
                pad_pid[:L] = lpid
            cnt = np.bincount(pad_pid[:L], minlength=NP).astype(np.float32)
            invcnt = np.zeros(NP, np.float32)
            nz = cnt > 0
            invcnt[nz] = 1.0 / cnt[nz]
            invcnt[NP - 1] = 0.0
            shards.append(dict(row=b, t0=t0, L=L, pid=pad_pid, invcnt=invcnt))
    return shards


def _make_meta(shards):
    """Union (over shards) of position-block <-> patch-block adjacency."""
    winb = [set() for _ in range(TB)]
    span = [set() for _ in range(NB)]
    for sh in shards:
        pp = sh["pid"]
        for tb in range(TB):
            blk = pp[tb * 128:(tb + 1) * 128]
            for nb in range(int(blk.min()) // 128, int(blk.max()) // 128 + 1):
                winb[tb].add(nb)
                span[nb].add(tb)
    for nb in range(NB):
        s = span[nb]
        assert s and s == set(range(min(s), max(s) + 1)), \
            f"patch block {nb} has non-contiguous tb span {sorted(s)}"
    return {
        "winb": tuple(tuple(sorted(w)) for w in winb),
        "span": tuple((min(s), max(s)) for s in span),
    }


def prepare_in_maps(byte_repr, Wq, bq, Wk, bk, Wv, bv, Wo, bo, Wv2, bv2,
                    Wo2, bo2, patch_boundaries):
    """Host-side sharding/marshalling: (shards, in_maps, flags, meta)."""
    byte_repr = np.asarray(byte_repr, np.float32)
    shards = _make_shards(patch_boundaries)
    meta = _make_meta(shards)
    Wo = np.asarray(Wo, np.float64)
    Wv2 = np.asarray(Wv2, np.float64)
    Wo2 = np.asarray(Wo2, np.float64)
    wfull = Wo2 @ (Wv2 @ Wo)
    bfull = (Wo2 @ (Wv2 @ np.asarray(bo, np.float64)
                    + np.asarray(bv2, np.float64))
             + np.asarray(bo2, np.float64))
    flags = {
        "bq": bool(np.any(np.asarray(bq))),
        "bk": bool(np.any(np.asarray(bk))),
        "bv": bool(np.any(np.asarray(bv))),
        "bfull": bool(np.any(bfull)),
    }
    wqT = np.ascontiguousarray(np.asarray(Wq, np.float32).T.astype(np.float16))
    wkT = np.ascontiguousarray(np.asarray(Wk, np.float32).T.astype(np.float16))
    wvT = np.ascontiguousarray(np.asarray(Wv, np.float32).T.astype(np.float16))
    wfullT = np.ascontiguousarray(wfull.T.astype(np.float16))
    iota_np = np.arange(NP, dtype=np.float32)
    iota_col = (np.arange(128, dtype=np.float32)[:, None]
                + 128.0 * np.arange(NB, dtype=np.float32)[None, :])
    iota_col = np.ascontiguousarray(iota_col)

    in_maps = []
    for sh in shards:
        xc = np.zeros((P, D), np.float16)
        if sh["L"]:
            xc[:sh["L"]] = byte_repr[sh["row"],
                                     sh["t0"]:sh["t0"] + sh["L"]].astype(
                                         np.float16)
        m = {
            "x16": xc,
            "xT16": np.ascontiguousarray(xc.T),
            "pid": sh["pid"].astype(np.float32),
            "iota_np": iota_np,
            "iota_col": iota_col,
            "invcnt": sh["invcnt"],
            "wqT": wqT, "wkT": wkT, "wvT": wvT, "wfullT": wfullT,
        }
        if flags["bq"]:
            m["bq"] = np.asarray(bq, np.float32)
        if flags["bk"]:
            m["bk"] = np.asarray(bk, np.float32)
        if flags["bv"]:
            m["bv"] = np.asarray(bv, np.float32)
        if flags["bfull"]:
            m["bfull"] = bfull.astype(np.float32)
        in_maps.append(m)
    return shards, in_maps, flags, meta


def kernel(byte_repr, Wq, bq, Wk, bk, Wv, bv, Wo, bo, Wv2, bv2, Wo2, bo2,
           patch_boundaries):
    shards, in_maps, flags, meta = prepare_in_maps(
        byte_repr, Wq, bq, Wk, bk, Wv, bv, Wo, bo, Wv2, bv2, Wo2, bo2,
        patch_boundaries)
    nc = get_program(flags, meta)
    res = bass_utils.run_bass_kernel_spmd(nc, in_maps, list(range(N_CORES)))
    out = np.zeros((B, S, D), np.float32)
    for sh, r in zip(shards, res.results):
        if sh["L"]:
            out[sh["row"], sh["t0"]:sh["t0"] + sh["L"]] = (
                r["out"][:sh["L"]].astype(np.float32))
    return out


# revision 17
# speedup vs baseline: 2.9256x; 1.2786x over previous
"""Trainium2 Bass kernel for nn_CrossAttentionPositionBridge.

Contract: kernel(**inputs) takes FULL unsharded inputs (as produced by
setup_inputs) and returns the FULL (4, 4096, 1024) float32 output.

Strategy (v2):
  - Each of the 4 rows is split at the first patch boundary >= 2048 into two
    chunks -> 8 chunks, one per NeuronCore.  Splitting at a patch boundary
    makes every patch fully contained in one chunk.  Chunks are zero-padded
    to P=2176 positions; local patch ids are padded with NP-1=383 (a dummy
    patch that only padded positions reference).
  - Ragged segment ops are matmuls against 0/1 selection matrices generated
    on-device with is_equal.  Patch ids are MONOTONE in position, so each
    128-position block only touches 1-2 of the three 128-patch blocks and
    each patch block only receives from a contiguous range of position
    blocks.  The host computes these windows (unioned over the 8 shards) and
    the kernel only emits the non-trivially-zero sel matmuls -- cutting the
    selection-matmul work to ~43% of the dense version.
  - All matmul operands are float16 (1 cycle/row on the PE at any free size;
    ints <= 2048 exact so the 0/1 sel masks and pid comparisons are exact).
    PSUM accumulation stays fp32; softmax statistics stay fp32 on the DVE.
    exp(score) values live in [e^-3, e^3] for this data regime, and
    1/denom is scaled by 256 with eps=0.01 so fp16 never over/underflows.
  - decode stage folded on host: out = gather(patch_heads @ (Wo2@Wv2@Wo).T).
  - x and the per-position k are never revisited: v and all gathered
    operands are kept SBUF-resident in fp16.
"""

import numpy as np

import concourse.bass as bass
import concourse.mybir as mybir
import concourse.tile as tile
from concourse import bacc, bass_utils
from concourse.bass import ts

B, S, D, H = 4, 4096, 1024, 16
HD = D // H
P = 2176           # padded chunk length
TB = P // 128      # 17 position blocks
NP = 384           # padded patch count
NB = NP // 128     # 3 patch blocks
DC = D // 128      # 8 feature chunks
N_CORES = 8

F32 = mybir.dt.float32
F16 = mybir.dt.float16
F8 = mybir.dt.float8e4

_PROG_CACHE = {}


def _build_body(nc, tc, aps, flags, meta, dbg=None, stages=frozenset(("head", "s4", "s67"))):
    """Emit the per-core kernel body into the TileContext."""
    from contextlib import ExitStack

    def dump(name, tile_ap):
        if dbg is not None and name in dbg:
            nc.sync.dma_start(dbg[name], tile_ap)

    winb = meta["winb"]          # tuple[tb] -> tuple of nb blocks touched
    span = meta["span"]          # tuple[nb] -> (tb_lo, tb_hi) inclusive

    x16, xT16, pid, iota_np, iota_col, invcnt = (
        aps["x16"], aps["xT16"], aps["pid"], aps["iota_np"], aps["iota_col"],
        aps["invcnt"])
    wqT, wkT, wvT, wfullT = aps["wqT"], aps["wkT"], aps["wvT"], aps["wfullT"]
    out = aps["out"]
    with_bq, with_bk, with_bv, with_bfull = (
        flags["bq"], flags["bk"], flags["bv"], flags["bfull"])

    x_r = x16.rearrange("(tb p) d -> p tb d", p=128)
    xT_r = xT16.rearrange("(dc p) t -> p dc t", p=128)
    xT8_r = aps["xT8"].rearrange("(dc p) t -> p dc t", p=128)
    pid_nat_r = pid.rearrange("(tb p) -> p tb", p=128)
    out_r = out.rearrange("(tb p) d -> p tb d", p=128)
    wq_r = wqT.rearrange("(dc p) d -> p dc d", p=128)
    wk_r = wkT.rearrange("(dc p) d -> p dc d", p=128)
    wv_r = wvT.rearrange("(dc p) d -> p dc d", p=128)
    wfull_r = wfullT.rearrange("(dc p) d -> p dc d", p=128)

    EQ = mybir.AluOpType.is_equal
    ADD = mybir.AluOpType.add
    MUL = mybir.AluOpType.mult

    with ExitStack() as ctx:
        # ---- long-lived tiles -------------------------------------------
        perm = ctx.enter_context(tc.tile_pool(name="perm", bufs=1))
        x_sb = perm.tile([128, TB, D], F16)          # resident x
        v_sb = perm.tile([128, TB, D], F16)          # resident v
        st_sb = perm.tile([128, TB, NP], F16)        # sel^T (t-part)
        sel_sb = perm.tile([128, NB, TB, 128], F16)  # sel (np-part)
        q_sb = perm.tile([128, NB, D], F16)
        qmT_sb = perm.tile([128, DC, NP], F16)
        upw_sb = perm.tile([128, DC, NP], F16)
        o2_sb = perm.tile([128, NB, D], F16)
        p16_sb = perm.tile([128, TB, H], F16)
        score_sb = perm.tile([128, TB, H], F32)
        invd16_sb = perm.tile([16, NP], F16)
        invcnt_repl = perm.tile([128, NP], F32)
        iota_np_repl = perm.tile([128, NP], F32)
        iota_col_sb = perm.tile([128, NB], F32)
        pid_nat = perm.tile([128, TB], F32)

        if dbg is not None:
            # debug dumps read whole tiles; zero the sparsely-written ones
            nc.gpsimd.memset(sel_sb[:], 0.0)
        nc.sync.dma_start(invcnt_repl[:], invcnt.partition_broadcast(128))
        nc.sync.dma_start(iota_np_repl[:], iota_np.partition_broadcast(128))
        nc.sync.dma_start(iota_col_sb[:], iota_col[:])
        nc.sync.dma_start(pid_nat[:], pid_nat_r[:])

        # x streamed per tb on the SP ring (17 slices into the perm tile)
        for tb in range(TB):
            nc.sync.dma_start(x_sb[:, tb, :], x_r[:, tb, :])

        # bias broadcasts (rarely used; zero biases skip these)
        bq_repl = bk_repl = bv_repl = bfull_repl = None
        if with_bq:
            bq_repl = perm.tile([128, D], F32)
            nc.sync.dma_start(bq_repl[:], aps["bq"].partition_broadcast(128))
        if with_bk:
            bk_repl = perm.tile([128, D], F32)
            nc.sync.dma_start(bk_repl[:], aps["bk"].partition_broadcast(128))
        if with_bv:
            bv_repl = perm.tile([128, D], F32)
            nc.sync.dma_start(bv_repl[:], aps["bv"].partition_broadcast(128))
        if with_bfull:
            bfull_repl = perm.tile([128, D], F32)
            nc.sync.dma_start(bfull_repl[:],
                              aps["bfull"].partition_broadcast(128))

        # ---- sel generation (DVE, fp16 out) -----------------------------
        with tc.tile_pool(name="pidr", bufs=1) as pidr:
            pid_repl = pidr.tile([128, P], F32)
            nc.sync.dma_start(pid_repl[:], pid.partition_broadcast(128))
            for tb in range(TB):
                nc.vector.tensor_tensor(
                    st_sb[:, tb, :],
                    pid_nat[:, tb:tb + 1].to_broadcast([128, NP]),
                    iota_np_repl[:], EQ)
            for tb in range(TB):
                for nb in winb[tb]:
                    nc.vector.tensor_tensor(
                        sel_sb[:, nb, tb, :],
                        iota_col_sb[:, nb:nb + 1].to_broadcast([128, 128]),
                        pid_repl[:, ts(tb, 128)], EQ)

            # ---- head: qm (windowed groups) + q-projection --------------
            with tc.tile_pool(name="wqp", bufs=1) as wqp, \
                 tc.tile_pool(name="psh", bufs=2, space="PSUM") as psh, \
                 tc.tile_pool(name="psq", bufs=2, space="PSUM") as psq:
                wq_sb = wqp.tile([128, DC, D], F16)
                nc.scalar.dma_start(wq_sb[:], wq_r[:])
                for nb in range(NB):
                    lo, hi = span[nb]
                    g0 = psh.tile([128, 512], F32, tag="qmg0")
                    g1 = psh.tile([128, 512], F32, tag="qmg1")
                    gt = (g0, g1)
                    for tb in range(lo, hi + 1):
                        for db in range(DC):
                            # one accumulation group per PSUM bank: start only
                            # on the bank's first write (pending-zero makes
                            # the other quarters' first writes overwrite)
                            nc.tensor.matmul(
                                gt[db // 4][:, ts(db % 4, 128)],
                                x_sb[:, tb, ts(db, 128)],
                                st_sb[:, tb, ts(nb, 128)],
                                start=(tb == lo and db % 4 == 0),
                                stop=(tb == hi and db % 4 == 3))
                    for db in range(DC):
                        nc.vector.tensor_mul(
                            qmT_sb[:, db, ts(nb, 128)],
                            gt[db // 4][:, ts(db % 4, 128)],
                            invcnt_repl[:, ts(nb, 128)])
                    for hf in range(2):
                        q_ps = psq.tile([128, 512], F32, tag="q")
                        for db in range(DC):
                            nc.tensor.matmul(
                                q_ps[:], qmT_sb[:, db, ts(nb, 128)],
                                wq_sb[:, db, ts(hf, 512)],
                                start=(db == 0), stop=(db == DC - 1))
                        dst = q_sb[:, nb, ts(hf, 512)]
                        if with_bq:
                            nc.vector.tensor_tensor(
                                dst, q_ps[:], bq_repl[:, ts(hf, 512)], ADD)
                        else:
                            nc.scalar.copy(dst, q_ps[:])

        dump("qmT", qmT_sb[:])
        dump("q", q_sb[:])

        # ---- S4': k, v, scores, p, dn, w', patch-head accumulation ------
        if "s4" not in stages:
            return
        invd_dram = aps["invd_rt"]
        wfp = ctx.enter_context(tc.tile_pool(name="wfp", bufs=1))
        wfull_sb = wfp.tile([128, DC, D], F16)
        nc.sync.dma_start(wfull_sb[:], wfull_r[:])
        with tc.tile_pool(name="wkv", bufs=1) as wkv, \
             tc.tile_pool(name="xts", bufs=3) as xts, \
             tc.tile_pool(name="zs", bufs=3) as zs, \
             tc.tile_pool(name="qps", bufs=2) as qps, \
             tc.tile_pool(name="ws", bufs=2) as ws, \
             tc.tile_pool(name="psmm", bufs=3, space="PSUM") as psmm, \
             tc.tile_pool(name="psup", bufs=2, space="PSUM") as psup, \
             tc.tile_pool(name="psdn", bufs=1, space="PSUM") as psdn:
            wk_sb = wkv.tile([128, DC, D], F8)
            wv_sb = wkv.tile([128, DC, D], F16)
            nc.scalar.dma_start(wk_sb[:], wk_r[:])
            nc.scalar.dma_start(wv_sb[:], wv_r[:])
            dn_ps = psdn.tile([16, NP], F32, name="dn_ps")
            dn_first = (0, winb[0][0])
            dn_last = (TB - 1, winb[TB - 1][-1])
            upgrp = {}

            def emit_qkv(hf, tb):
                """PE: qp gather, k, v; chain: z -> score -> exp(p16)."""
                wtb = winb[tb]
                qp_ps = psmm.tile([128, 512], F32, tag="mm",
                                  name=f"qp{hf}_{tb}")
                for j, nb in enumerate(wtb):
                    nc.tensor.matmul(
                        qp_ps[:], sel_sb[:, nb, tb, :],
                        q_sb[:, nb, ts(hf, 512)],
                        start=(j == 0), stop=(j == len(wtb) - 1))
                qp_t = qps.tile([128, 512], F32, tag="qp")
                nc.scalar.copy(qp_t[:], qp_ps[:])

                xt_t = xts.tile([128, DC, 128], F16, tag="xt")
                nc.scalar.dma_start(xt_t[:], xT_r[:, :, ts(tb, 128)])
                xt8_t = xts.tile([128, DC, 128], F8, tag="xt8")
                nc.scalar.dma_start(xt8_t[:], xT8_r[:, :, ts(tb, 128)])

                # fp8 DoubleRow: two 128-deep k-tiles per matmul
                k_ps = psmm.tile([128, 512], F32, tag="mm",
                                 name=f"k{hf}_{tb}")
                for dp in range(DC // 2):
                    nc.tensor.matmul(
                        k_ps[:], xt8_t[:, 2 * dp:2 * dp + 2, :],
                        wk_sb[:, 2 * dp:2 * dp + 2, ts(hf, 512)],
                        start=(dp == 0), stop=(dp == DC // 2 - 1),
                        perf_mode=mybir.MatmulPerfMode.DoubleRow)
                z_t = zs.tile([128, 512], F32, tag="z")
                if with_bk:
                    nc.vector.tensor_tensor(
                        z_t[:], k_ps[:], bk_repl[:, ts(hf, 512)], ADD)
                    nc.vector.tensor_mul(z_t[:], z_t[:], qp_t[:])
                else:
                    nc.vector.tensor_mul(z_t[:], k_ps[:], qp_t[:])
                nc.vector.tensor_reduce(
                    score_sb[:, tb, ts(hf, 8)],
                    z_t[:].rearrange("p (h e) -> p h e", e=HD),
                    mybir.AxisListType.X, ADD)
                nc.scalar.activation(
                    p16_sb[:, tb, ts(hf, 8)], score_sb[:, tb, ts(hf, 8)],
                    mybir.ActivationFunctionType.Exp,
                    scale=1.0 / float(HD) ** 0.5)

                v_ps = psmm.tile([128, 512], F32, tag="mm",
                                 name=f"v{hf}_{tb}")
                for db in range(DC):
                    nc.tensor.matmul(
                        v_ps[:], xt_t[:, db, :], wv_sb[:, db, ts(hf, 512)],
                        start=(db == 0), stop=(db == DC - 1))
                if with_bv:
                    nc.vector.tensor_tensor(
                        v_sb[:, tb, ts(hf, 512)], v_ps[:],
                        bv_repl[:, ts(hf, 512)], ADD)
                else:
                    nc.scalar.copy(v_sb[:, tb, ts(hf, 512)], v_ps[:])

            def emit_wup(hf, tb):
                """Lagged: w' = p16*v (unnormalized attn), patch-head accum;
                dn in the hf1 pass."""
                wtb = winb[tb]
                w_t = ws.tile([128, 8, HD], F16, tag="w")
                nc.vector.tensor_tensor(
                    w_t[:],
                    v_sb[:, tb, ts(hf, 512)].rearrange(
                        "p (h e) -> p h e", e=HD),
                    p16_sb[:, tb, ts(hf, 8), None].to_broadcast([128, 8, HD]),
                    MUL)
                w_f = w_t[:].rearrange("p h e -> p (h e)")
                for nb in wtb:
                    lo, hi = span[nb]
                    if tb == lo:
                        upgrp[(nb, hf)] = psup.tile(
                            [128, 512], F32, tag=f"up{hf}",
                            name=f"up{hf}_{nb}")
                    gt = upgrp[(nb, hf)]
                    for dq in range(4):
                        nc.tensor.matmul(
                            gt[:, ts(dq, 128)], w_f[:, ts(dq, 128)],
                            st_sb[:, tb, ts(nb, 128)],
                            start=(tb == lo and dq == 0),
                            stop=(tb == hi and dq == 3))
                    if tb == hi:
                        # raw drain (normalization applied later via M)
                        for dq in range(4):
                            nc.vector.tensor_copy(
                                upw_sb[:, hf * 4 + dq, ts(nb, 128)],
                                gt[:, ts(dq, 128)])
                if hf == 1:
                    for nb in wtb:
                        # transposed denominator: out [16 heads, 128 patches]
                        nc.tensor.matmul(
                            dn_ps[:, ts(nb, 128)], p16_sb[:, tb, :],
                            st_sb[:, tb, ts(nb, 128)],
                            start=((tb, nb) == dn_first),
                            stop=((tb, nb) == dn_last))

            LAG = 2
            for hf in range(2):
                for tb in range(TB):
                    emit_qkv(hf, tb)
                    if tb >= LAG:
                        emit_wup(hf, tb - LAG)
                for tb in range(TB - LAG, TB):
                    emit_wup(hf, tb)

            # invd = clamp(1/dn), head-major -> tiny DRAM bounce for the
            # per-head broadcast reload in the tail
            dn_t = zs.tile([16, NP], F32, tag="dn")
            nc.vector.tensor_scalar_add(dn_t[:], dn_ps[:], 1e-30)
            inv_t = zs.tile([16, NP], F32, tag="inv")
            nc.vector.reciprocal(inv_t[:], dn_t[:])
            # clamp so empty-patch 1/eps stays fp16-finite
            nc.vector.tensor_scalar_min(invd16_sb[:], inv_t[:], 60000.0)
            nc.sync.dma_start(invd_dram, invd16_sb[:])

        dump("score", score_sb[:])
        dump("p16", p16_sb[:])
        dump("invd16", invd16_sb[:])
        dump("v", v_sb[:])
        dump("st", st_sb[:])
        dump("sel", sel_sb[:])

        # ---- tail: normalize patch heads, o2, per-position gather -------
        if "s67" not in stages:
            return
        with tc.tile_pool(name="pso", bufs=2, space="PSUM") as pso, \
             tc.tile_pool(name="oc", bufs=4) as oc:
            m_sb = wfp.tile([128, DC, NP], F16)
            for db in range(DC):
                for u in range(2):
                    nc.sync.dma_start(
                        m_sb[:][ts(u, 64), db, :],
                        invd_dram[2 * db + u].partition_broadcast(64))
            for db in range(DC):
                nc.vector.tensor_mul(upw_sb[:, db, :], upw_sb[:, db, :],
                                     m_sb[:, db, :])

            dump("upw", upw_sb[:])

            done_o2 = -1
            for nb in range(NB):
                for hf in range(2):
                    o2_ps = pso.tile([128, 512], F32, tag="o2")
                    for db in range(DC):
                        nc.tensor.matmul(
                            o2_ps[:], upw_sb[:, db, ts(nb, 128)],
                            wfull_sb[:, db, ts(hf, 512)],
                            start=(db == 0), stop=(db == DC - 1))
                    dst = o2_sb[:, nb, ts(hf, 512)]
                    if with_bfull:
                        nc.vector.tensor_tensor(
                            dst, o2_ps[:], bfull_repl[:, ts(hf, 512)], ADD)
                    else:
                        nc.scalar.copy(dst, o2_ps[:])
                # emit gathers for all position blocks whose windows are ready
                for tb in range(TB):
                    if winb[tb][-1] != nb:
                        continue
                    wtb = winb[tb]
                    for hf in range(2):
                        o_ps = pso.tile([128, 512], F32, tag="o")
                        for j, nbb in enumerate(wtb):
                            nc.tensor.matmul(
                                o_ps[:], sel_sb[:, nbb, tb, :],
                                o2_sb[:, nbb, ts(hf, 512)],
                                start=(j == 0), stop=(j == len(wtb) - 1))
                        oc_t = oc.tile([128, 512], F16, tag="oc")
                        if hf == 0:
                            nc.vector.tensor_copy(oc_t[:], o_ps[:])
                            nc.sync.dma_start(out_r[:, tb, ts(hf, 512)],
                                              oc_t[:])
                        else:
                            nc.scalar.copy(oc_t[:], o_ps[:])
                            nc.scalar.dma_start(out_r[:, tb, ts(hf, 512)],
                                                oc_t[:])

        dump("o2", o2_sb[:])


def _build_program(flags, meta, loop_reps=None,
                   stages=frozenset(("head", "s4", "s67"))):
    nc = bacc.Bacc("TRN2", target_bir_lowering=False, debug=False)
    aps = {}
    aps["x16"] = nc.dram_tensor("x16", [P, D], F16, kind="ExternalInput").ap()
    aps["xT16"] = nc.dram_tensor("xT16", [D, P], F16,
                                 kind="ExternalInput").ap()
    aps["xT8"] = nc.dram_tensor("xT8", [D, P], F8,
                                kind="ExternalInput").ap()
    aps["pid"] = nc.dram_tensor("pid", [P], F32, kind="ExternalInput").ap()
    aps["iota_np"] = nc.dram_tensor("iota_np", [NP], F32,
                                    kind="ExternalInput").ap()
    aps["iota_col"] = nc.dram_tensor("iota_col", [128, NB], F32,
                                     kind="ExternalInput").ap()
    aps["invcnt"] = nc.dram_tensor("invcnt", [NP], F32,
                                   kind="ExternalInput").ap()
    for w in ("wqT", "wvT", "wfullT"):
        aps[w] = nc.dram_tensor(w, [D, D], F16, kind="ExternalInput").ap()
    aps["wkT"] = nc.dram_tensor("wkT", [D, D], F8, kind="ExternalInput").ap()
    aps["invd_rt"] = nc.dram_tensor("invd_rt", [16, NP], F16).ap()
    for b in ("bq", "bk", "bv", "bfull"):
        if flags[b]:
            aps[b] = nc.dram_tensor(b, [D], F32, kind="ExternalInput").ap()
    if loop_reps is not None:
        # Timing build: the big output stays in internal DRAM so the host
        # only ships a tiny donated zero buffer per timed call.
        aps["out"] = nc.dram_tensor("out_scratch", [P, D], F16).ap()
        dummy = nc.dram_tensor("out", [1, 1], F32, kind="ExternalOutput").ap()
    else:
        aps["out"] = nc.dram_tensor("out", [P, D], F16,
                                    kind="ExternalOutput").ap()

    with tile.TileContext(nc) as tc:
        if loop_reps is not None:
            with tc.For_i(0, loop_reps, 1):
                _build_body(nc, tc, aps, flags, meta, stages=stages)
            with tc.tile_pool(name="dum", bufs=1) as dum:
                d_t = dum.tile([1, 1], F32)
                nc.vector.memset(d_t[:], 0.0)
                nc.sync.dma_start(dummy[:], d_t[:])
        else:
            _build_body(nc, tc, aps, flags, meta, stages=stages)
    nc.compile()
    return nc


def get_program(flags=None, meta=None, loop_reps=None,
                stages=frozenset(("head", "s4", "s67"))):
    if flags is None:
        flags = {"bq": False, "bk": False, "bv": False, "bfull": False}
    key = (tuple(sorted(flags.items())), meta["winb"], meta["span"], loop_reps,
           stages)
    if key not in _PROG_CACHE:
        _PROG_CACHE[key] = _build_program(flags, meta, loop_reps, stages)
    return _PROG_CACHE[key]


def _make_shards(patch_boundaries):
    pb = np.asarray(patch_boundaries)
    shards = []
    for b in range(pb.shape[0]):
        bnd = (pb[b] != 0).astype(np.int64)
        pid = np.cumsum(bnd) - bnd[0]
        bpos = np.nonzero(bnd)[0]
        cand = bpos[bpos >= S // 2]
        split = int(cand[0]) if len(cand) else S
        for (t0, t1) in ((0, split), (split, S)):
            L = t1 - t0
            assert L <= P, f"chunk length {L} exceeds padded size {P}"
            pad_pid = np.full(P, NP - 1, np.int64)
            if L:
                lpid = pid[t0:t1] - pid[t0]
                assert lpid[-1] + 1 <= NP - 1, "too many patches in chunk"
                pad_pid[:L] = lpid
            cnt = np.bincount(pad_pid[:L], minlength=NP).astype(np.float32)
            invcnt = np.zeros(NP, np.float32)
            nz = cnt > 0
            invcnt[nz] = 1.0 / cnt[nz]
            invcnt[NP - 1] = 0.0
            shards.append(dict(row=b, t0=t0, L=L, pid=pad_pid, invcnt=invcnt))
    return shards


def _make_meta(shards):
    """Union (over shards) of position-block <-> patch-block adjacency."""
    winb = [set() for _ in range(TB)]
    span = [set() for _ in range(NB)]
    for sh in shards:
        pp = sh["pid"]
        for tb in range(TB):
            blk = pp[tb * 128:(tb + 1) * 128]
            for nb in range(int(blk.min()) // 128, int(blk.max()) // 128 + 1):
                winb[tb].add(nb)
                span[nb].add(tb)
    for nb in range(NB):
        s = span[nb]
        assert s and s == set(range(min(s), max(s) + 1)), \
            f"patch block {nb} has non-contiguous tb span {sorted(s)}"
    return {
        "winb": tuple(tuple(sorted(w)) for w in winb),
        "span": tuple((min(s), max(s)) for s in span),
    }


def prepare_in_maps(byte_repr, Wq, bq, Wk, bk, Wv, bv, Wo, bo, Wv2, bv2,
                    Wo2, bo2, patch_boundaries):
    """Host-side sharding/marshalling: (shards, in_maps, flags, meta)."""
    byte_repr = np.asarray(byte_repr, np.float32)
    shards = _make_shards(patch_boundaries)
    meta = _make_meta(shards)
    Wo = np.asarray(Wo, np.float64)
    Wv2 = np.asarray(Wv2, np.float64)
    Wo2 = np.asarray(Wo2, np.float64)
    wfull = Wo2 @ (Wv2 @ Wo)
    bfull = (Wo2 @ (Wv2 @ np.asarray(bo, np.float64)
                    + np.asarray(bv2, np.float64))
             + np.asarray(bo2, np.float64))
    flags = {
        "bq": bool(np.any(np.asarray(bq))),
        "bk": bool(np.any(np.asarray(bk))),
        "bv": bool(np.any(np.asarray(bv))),
        "bfull": bool(np.any(bfull)),
    }
    F8NP = mybir.dt.np(F8)
    wqT = np.ascontiguousarray(np.asarray(Wq, np.float32).T.astype(np.float16))
    wkT = np.ascontiguousarray(np.asarray(Wk, np.float32).T.astype(F8NP))
    wvT = np.ascontiguousarray(np.asarray(Wv, np.float32).T.astype(np.float16))
    wfullT = np.ascontiguousarray(wfull.T.astype(np.float16))
    iota_np = np.arange(NP, dtype=np.float32)
    iota_col = (np.arange(128, dtype=np.float32)[:, None]
                + 128.0 * np.arange(NB, dtype=np.float32)[None, :])
    iota_col = np.ascontiguousarray(iota_col)

    in_maps = []
    for sh in shards:
        xc = np.zeros((P, D), np.float16)
        if sh["L"]:
            xc[:sh["L"]] = byte_repr[sh["row"],
                                     sh["t0"]:sh["t0"] + sh["L"]].astype(
                                         np.float16)
        m = {
            "x16": xc,
            "xT16": np.ascontiguousarray(xc.T),
            "xT8": np.ascontiguousarray(xc.T.astype(mybir.dt.np(F8))),
            "pid": sh["pid"].astype(np.float32),
            "iota_np": iota_np,
            "iota_col": iota_col,
            "invcnt": sh["invcnt"],
            "wqT": wqT, "wkT": wkT, "wvT": wvT, "wfullT": wfullT,
        }
        if flags["bq"]:
            m["bq"] = np.asarray(bq, np.float32)
        if flags["bk"]:
            m["bk"] = np.asarray(bk, np.float32)
        if flags["bv"]:
            m["bv"] = np.asarray(bv, np.float32)
        if flags["bfull"]:
            m["bfull"] = bfull.astype(np.float32)
        in_maps.append(m)
    return shards, in_maps, flags, meta


def kernel(byte_repr, Wq, bq, Wk, bk, Wv, bv, Wo, bo, Wv2, bv2, Wo2, bo2,
           patch_boundaries):
    shards, in_maps, flags, meta = prepare_in_maps(
        byte_repr, Wq, bq, Wk, bk, Wv, bv, Wo, bo, Wv2, bv2, Wo2, bo2,
        patch_boundaries)
    nc = get_program(flags, meta)
    res = bass_utils.run_bass_kernel_spmd(nc, in_maps, list(range(N_CORES)))
    out = np.zeros((B, S, D), np.float32)
    for sh, r in zip(shards, res.results):
        if sh["L"]:
            out[sh["row"], sh["t0"]:sh["t0"] + sh["L"]] = (
                r["out"][:sh["L"]].astype(np.float32))
    return out
